# revision 2
# baseline (speedup 1.0000x reference)
"""LocalAttention (B=1, S=4096, D=1024, H=16, hd=64, window=128) on 8 trn2 cores.

Sequence-parallel: core c owns queries [512c, 512c+512), K/V halo of 768 rows.
All weights replicated bf16; fp32 PSUM accumulation.

v2 vs baseline:
  - attention processes 2 heads per "pack": one [128,768] fp32 PSUM score tile
    (2 banks; head A cols 0-383, head B 384-767), ONE exp per pack.
  - score r-chunks stored [r0|r2|r1] per head so the two masked triangle chunks
    are adjacent: ONE broadcast tensor_mul over [128,2,256] applies both heads'
    masks (middle all-ones chunk never multiplied).
  - softmax denominators via ones-column (VROW=65); per pack one DVE reciprocal
    [128,2] + one broadcast tensor_mul (PSUM->SBUF bf16) normalizes both heads.
  - software-pipelined flat pack loop: QK(pk) issues ahead, PV lags 2 packs so
    exp/mask latency never stalls the in-order PE queue; transpose + output
    projection of window t fill PE slack during packs 8t+10..8t+17.
  - 8 PE transposes of a q-block share one PSUM bank; ONE [128,8x128] copy
    evacuates them.
  - input DMAs ordered (qtin, per-eb wq slices first) so qproj starts ~3.5us.
"""

import os

import numpy as np
import ml_dtypes

import concourse.bass as bass
import concourse.bacc as bacc
import concourse.mybir as mybir
import concourse.tile as tile
from concourse.bass_utils import run_bass_kernel_spmd

BF16 = mybir.dt.bfloat16
FP32 = mybir.dt.float32

NCORES = 8
S = 4096
D = 1024
H = 16
HD = 64
E = H * HD  # 1024
WIN = 128
SL = S // NCORES       # 512 queries per core
SK = SL + 2 * WIN      # 768 keys/values incl. halo
NQB = SL // 128        # 4 query blocks
NKB = SK // 128        # 6 key blocks
NDB = D // 128         # 8 contraction blocks
NEB = E // 128         # 8 embed blocks
VROW = HD + 1          # 65: v columns per head incl. ones column
NPACK = NQB * NEB      # 32 flat (qb, head-pair) packs
PW = NPACK // NQB      # 8 packs per q-block window
ROFF = (0, 256, 128)   # col offset of r-chunk within a head's 384 block

_CACHE = {}
LAST_RESULT = None


def _build_nc():
    nc = bacc.Bacc("TRN2", target_bir_lowering=False, debug=False)

    qt_d = nc.dram_tensor("qt", [D, SL], BF16, kind="ExternalInput").ap()
    kt_d = nc.dram_tensor("kt", [D, SK], BF16, kind="ExternalInput").ap()
    vt_d = nc.dram_tensor("vt", [D, SK], BF16, kind="ExternalInput").ap()
    wq_d = nc.dram_tensor("wq", [D, E], BF16, kind="ExternalInput").ap()
    wk_d = nc.dram_tensor("wk", [D, E], BF16, kind="ExternalInput").ap()
    wv_d = nc.dram_tensor("wv", [D, E], BF16, kind="ExternalInput").ap()
    wo_d = nc.dram_tensor("wo", [E, D], BF16, kind="ExternalInput").ap()
    msk_d = nc.dram_tensor("msk", [NQB, 128, 256], BF16, kind="ExternalInput").ap()
    idn_d = nc.dram_tensor("idn", [128, 128], BF16, kind="ExternalInput").ap()
    out_d = nc.dram_tensor("out", [SL, D], FP32, kind="ExternalOutput").ap()

    with tile.TileContext(nc) as tc:
        pools = []

        def pool(name, bufs, **kw):
            p = tc.tile_pool(name=name, bufs=bufs, **kw)
            pools.append(p)
            return p.__enter__()

        const = pool("const", 1)
        ps_pool = pool("ps", 2, space="PSUM")        # 2 banks: proj/oproj/tr
        pscore = pool("pscore", 4, space="PSUM")     # 4x1 bank: score half-packs
        ppv_pool = pool("ppv", 2, space="PSUM")      # 2 banks: pv packs
        ep = pool("expp", 4)
        rp = pool("recip", 4)

        # ---- persistent SBUF tensors ----
        wq_sb = const.tile([128, NDB * E], BF16, tag="wq")
        wk_sb = const.tile([128, NDB * E], BF16, tag="wk")
        wv_sb = const.tile([128, NDB * E], BF16, tag="wv")
        wo_sb = const.tile([128, NEB * D], BF16, tag="wo")
        qtin_sb = const.tile([128, NDB * SL], BF16, tag="qtin")
        ktin_sb = const.tile([128, NDB * SK], BF16, tag="ktin")
        vtin_sb = const.tile([128, NDB * SK], BF16, tag="vtin")
        qt_sb = const.tile([128, NEB * SL], BF16, tag="qt")    # [e,s] per e-blk
        kt_sb = const.tile([128, NEB * SK], BF16, tag="kt")
        v_sb = const.tile([128, NKB * H * VROW], BF16, tag="v")  # [s, h*65] per k-blk
        msk_sb = const.tile([128, NQB * 256], BF16, tag="msk")
        idn_sb = const.tile([128, 128], BF16, tag="idn")
        ao_sb = const.tile([128, NQB * E], BF16, tag="ao")     # attn out [sq, e]
        aot_sb = const.tile([128, NEB * SL], BF16, tag="aot")  # transposed [e, sq]
        o_sb = const.tile([128, NQB * D], FP32, tag="o")

        sync = nc.sync

        # ---- input DMAs: whole tensors on the sync queue, dependency order.
        # (eb-sliced loads produce 256B descriptors, ~6x slower; and extra
        # dma_starts serialize on the limited DMA semaphore slots.)
        sync.dma_start(
            qtin_sb[:].rearrange("p (b e) -> p b e", e=SL),
            qt_d.rearrange("(b p) e -> p b e", p=128),
        )
        sync.dma_start(
            wq_sb[:].rearrange("p (b e) -> p b e", e=E),
            wq_d.rearrange("(b p) e -> p b e", p=128),
        )
        sync.dma_start(
            vtin_sb[:].rearrange("p (b e) -> p b e", e=SK),
            vt_d.rearrange("(b p) e -> p b e", p=128),
        )
        sync.dma_start(
            wv_sb[:].rearrange("p (b e) -> p b e", e=E),
            wv_d.rearrange("(b p) e -> p b e", p=128),
        )
        sync.dma_start(
            ktin_sb[:].rearrange("p (b e) -> p b e", e=SK),
            kt_d.rearrange("(b p) e -> p b e", p=128),
        )
        sync.dma_start(
            wk_sb[:].rearrange("p (b e) -> p b e", e=E),
            wk_d.rearrange("(b p) e -> p b e", p=128),
        )
        sync.dma_start(
            wo_sb[:].rearrange("p (b e) -> p b e", e=D),
            wo_d.rearrange("(b p) e -> p b e", p=128),
        )
        # small late-needed tensors on the scalar queue
        nc.scalar.dma_start(
            msk_sb[:].rearrange("p (m c) -> p m c", c=256),
            msk_d.rearrange("m p c -> p m c"),
        )
        nc.scalar.dma_start(idn_sb[:], idn_d[:])

        # ones columns of v_sb (col hd=64 of each head group)
        v3 = v_sb[:].rearrange("p (k h c) -> p k h c", k=NKB, h=H)
        nc.gpsimd.memset(v3[:, :, :, HD:VROW], 1.0)

        # ---- q projection: qt[e,s] = Wq[d,e].T @ QT[d,s] ----
        for eb in range(NEB):
            ps = ps_pool.tile([128, 512], FP32, tag="ps", name=f"psq{eb}")
            for db in range(NDB):
                nc.tensor.matmul(
                    ps[:],
                    lhsT=wq_sb[:, db * E + eb * 128: db * E + (eb + 1) * 128],
                    rhs=qtin_sb[:, db * SL: db * SL + SL],
                    start=(db == 0),
                    stop=(db == NDB - 1),
                )
            nc.vector.tensor_copy(qt_sb[:, eb * SL:(eb + 1) * SL], ps[:])

        # ---- k projection for one e-block (two s-chunks) ----
        def emit_kproj_eb(eb):
            for s0, s1 in ((0, 512), (512, SK)):
                ps = ps_pool.tile([128, 512], FP32, tag="ps", name=f"psk{eb}_{s0}")
                for db in range(NDB):
                    nc.tensor.matmul(
                        ps[:, : s1 - s0],
                        lhsT=wk_sb[:, db * E + eb * 128: db * E + (eb + 1) * 128],
                        rhs=ktin_sb[:, db * SK + s0: db * SK + s1],
                        start=(db == 0),
                        stop=(db == NDB - 1),
                    )
                if s1 - s0 == 512:
                    nc.vector.tensor_copy(
                        kt_sb[:, eb * SK + s0: eb * SK + s1], ps[:, : s1 - s0]
                    )
                else:
                    nc.scalar.copy(
                        kt_sb[:, eb * SK + s0: eb * SK + s1], ps[:, : s1 - s0]
                    )

        # ---- v projection (natural) for one (kb, eh) group ----
        def emit_vproj_group(kb, eh):
            ps = ps_pool.tile([128, 512], FP32, tag="ps", name=f"psv{kb}_{eh}")
            for db in range(NDB):
                nc.tensor.matmul(
                    ps[:],
                    lhsT=vtin_sb[:, db * SK + kb * 128: db * SK + (kb + 1) * 128],
                    rhs=wv_sb[:, db * E + eh * 512: db * E + (eh + 1) * 512],
                    start=(db == 0),
                    stop=(db == NDB - 1),
                )
            dst = v3[:, kb, eh * 8:(eh + 1) * 8, 0:HD]
            src = ps[:].rearrange("p (h c) -> p h c", c=HD)
            nc.scalar.copy(dst, src)

        # v for the first attention window (kb 0..2) before the pack loop;
        # kb 3/4/5 are woven into windows 0/1/2 as PE filler.
        for kb in range(3):
            for eh in range(2):
                emit_vproj_group(kb, eh)

        # ---- attention: flat pack loop, PE software-pipelined ----
        scale = 1.0 / np.sqrt(HD)
        expps = {}
        ppvs = {}
        state = {}

        def emit_qk_exp(pk):
            qb, p = divmod(pk, PW)
            expp = ep.tile([128, 768], BF16, tag="expp", name=f"ex{pk}")
            for hcol, hp in ((0, 0), (1, 64)):
                base = hcol * 384
                pscr = pscore.tile(
                    [128, 384], FP32, tag="scr", name=f"scr{pk}_{hcol}"
                )
                kh = kt_sb[hp:hp + HD]
                qh = qt_sb[hp:hp + HD]
                for r in range(3):
                    kb = qb + r
                    nc.tensor.matmul(
                        pscr[:, ROFF[r]: ROFF[r] + 128],
                        lhsT=kh[:, p * SK + kb * 128: p * SK + (kb + 1) * 128],
                        rhs=qh[:, p * SL + qb * 128: p * SL + (qb + 1) * 128],
                        start=True,
                        stop=True,
                    )
                nc.scalar.activation(
                    expp[:, base: base + 384],
                    pscr[:],
                    mybir.ActivationFunctionType.Exp,
                    scale=scale,
                )
            expps[pk] = expp

        def emit_mask(pk):
            qb = pk // PW
            e3 = expps[pk][:].rearrange("p (h c) -> p h c", c=384)[:, :, 0:256]
            m = msk_sb[:, qb * 256:(qb + 1) * 256]
            mb = m.unsqueeze(1).broadcast_to((128, 2, 256))
            nc.vector.tensor_mul(e3, e3, mb)

        def emit_pv(pk):
            qb, p = divmod(pk, PW)
            expp = expps.pop(pk)
            ppv = ppv_pool.tile([128, 2 * VROW], FP32, tag="pv", name=f"pv{pk}")
            for hcol in range(2):
                h = 2 * p + hcol
                for r in range(3):
                    kb = qb + r
                    nc.tensor.matmul(
                        ppv[:, hcol * VROW:(hcol + 1) * VROW],
                        lhsT=expp[
                            :, hcol * 384 + ROFF[r]: hcol * 384 + ROFF[r] + 128
                        ],
                        rhs=v_sb[:, (kb * H + h) * VROW:(kb * H + h + 1) * VROW],
                        start=(r == 0),
                        stop=(r == 2),
                    )
            ppvs[pk] = ppv

        def emit_norm(pk):
            qb, p = divmod(pk, PW)
            ppv = ppvs.pop(pk)
            ppv3 = ppv[:].rearrange("p (h c) -> p h c", c=VROW)
            rcp = rp.tile([128, 2], FP32, tag="rcp", name=f"rcp{pk}")
            nc.vector.reciprocal(rcp[:], ppv3[:, :, HD])
            ao3 = ao_sb[
                :, qb * E + 2 * p * HD: qb * E + (2 * p + 2) * HD
            ].rearrange("p (h c) -> p h c", c=HD)
            rcp_b = rcp[:].unsqueeze(2).broadcast_to((128, 2, HD))
            nc.vector.tensor_mul(ao3, ppv3[:, :, 0:HD], rcp_b)

        def emit_oproj_mm(t, dh, ebs, slot_key):
            if (t, dh) not in state:
                state[(t, dh)] = ps_pool.tile(
                    [128, 512], FP32, tag="ps", name=f"po{t}_{dh}"
                )
            po = state[(t, dh)]
            for eb in ebs:
                nc.tensor.matmul(
                    po[:],
                    lhsT=aot_sb[:, eb * SL + t * 128: eb * SL + (t + 1) * 128],
                    rhs=wo_sb[:, eb * D + dh * 512: eb * D + (dh + 1) * 512],
                    start=(eb == 0),
                    stop=(eb == NEB - 1),
                )
            if ebs[-1] == NEB - 1:
                po = state.pop((t, dh))
                nc.vector.tensor_copy(
                    o_sb[:, t * D + dh * 512: t * D + (dh + 1) * 512], po[:]
                )

        def emit_filler(pk):
            """Fillers for target window t occupy packs 8t+10 .. 8t+17."""
            if pk < 10:
                return
            t, u = divmod(pk - 10, PW)
            if t >= NQB:
                return
            if u == 0 or u == 1:
                # 3 transposes each into the shared pt bank
                if u == 0:
                    state[("pt", t)] = ps_pool.tile(
                        [128, 1024], BF16, tag="ps", name=f"pt{t}"
                    )
                pt = state[("pt", t)]
                for eb in (range(0, 3) if u == 0 else range(3, 6)):
                    nc.tensor.transpose(
                        pt[:, eb * 128:(eb + 1) * 128],
                        ao_sb[:, t * E + eb * 128: t * E + (eb + 1) * 128],
                        idn_sb[:],
                    )
            elif u == 2:
                pt = state.pop(("pt", t))
                for eb in (6, 7):
                    nc.tensor.transpose(
                        pt[:, eb * 128:(eb + 1) * 128],
                        ao_sb[:, t * E + eb * 128: t * E + (eb + 1) * 128],
                        idn_sb[:],
                    )
                # one evacuation of all 8 transposed chunks
                nc.scalar.copy(
                    aot_sb[:]
                    .rearrange("p (b s) -> p b s", s=SL)[:, :, t * 128:(t + 1) * 128],
                    pt[:].rearrange("p (b s) -> p b s", s=128),
                )
            elif u == 3:
                emit_oproj_mm(t, 0, [0, 1, 2, 3], None)
            elif u == 4:
                emit_oproj_mm(t, 0, [4, 5, 6, 7], None)
            elif u == 5:
                emit_oproj_mm(t, 1, [0, 1, 2, 3], None)
            elif u == 6:
                emit_oproj_mm(t, 1, [4, 5, 6, 7], None)
            elif u == 7:
                sync.dma_start(
                    out_d[t * 128:(t + 1) * 128, :], o_sb[:, t * D:(t + 1) * D]
                )

        for pk in range(NPACK + 2):
            qb, p = divmod(pk, PW)
            if pk < NPACK:
                if qb == 0:
                    # kproj for this pack's head-pair, right before its QK
                    emit_kproj_eb(p)
                emit_qk_exp(pk)
            if pk >= 2 and pk - 2 < NPACK:
                emit_pv(pk - 2)
                emit_norm(pk - 2)
            if pk >= 1 and pk - 1 < NPACK:
                emit_mask(pk - 1)
            # weave remaining vproj groups: kb=qb+3, one group at p=1 and p=5
            if pk < NPACK and qb < 3 and p in (1, 5):
                emit_vproj_group(qb + 3, 0 if p == 1 else 1)
            emit_filler(pk)

        # tail: remaining fillers (windows whose slots extend past NPACK+1)
        for pk in range(NPACK + 2, NQB * PW + 10 + PW):
            emit_filler(pk)

        for p in reversed(pools):
            p.__exit__(None, None, None)

    nc.compile()
    return nc


def _host_inputs(query, key, value, Wq, Wk, Wv, Wo):
    bf = ml_dtypes.bfloat16
    q2 = np.ascontiguousarray(query.reshape(S, D))
    k2 = np.asarray(key).reshape(S, D)
    v2 = np.asarray(value).reshape(S, D)
    kpad = np.zeros((S + 2 * WIN, D), np.float32)
    kpad[WIN:WIN + S] = k2
    vpad = np.zeros((S + 2 * WIN, D), np.float32)
    vpad[WIN:WIN + S] = v2

    wq = np.ascontiguousarray(Wq.astype(bf))
    wk = np.ascontiguousarray(Wk.astype(bf))
    wv = np.ascontiguousarray(Wv.astype(bf))
    wo = np.ascontiguousarray(Wo.astype(bf))
    idn = np.eye(128, dtype=bf)

    kt = np.arange(128)[:, None]
    qi = np.arange(128)[None, :]
    tri0 = (qi <= kt).astype(bf)
    tri2 = (kt <= qi).astype(bf)
    zeros = np.zeros((128, 128), bf)

    in_maps = []
    for c in range(NCORES):
        s0 = c * SL
        qt = np.ascontiguousarray(q2[s0:s0 + SL].T.astype(bf))
        ktc = np.ascontiguousarray(kpad[s0:s0 + SK].T.astype(bf))
        vtc = np.ascontiguousarray(vpad[s0:s0 + SK].T.astype(bf))
        msk = np.empty((NQB, 128, 256), bf)
        for qb in range(NQB):
            m0 = zeros if (c == 0 and qb == 0) else tri0
            m2 = zeros if (c == NCORES - 1 and qb == NQB - 1) else tri2
            msk[qb] = np.hstack([m0, m2])
        in_maps.append({
            "qt": qt, "kt": ktc, "vt": vtc,
            "wq": wq, "wk": wk, "wv": wv, "wo": wo,
            "msk": msk, "idn": idn,
        })
    return in_maps


def kernel(query, key, value, Wq, Wk, Wv, Wo):
    global LAST_RESULT
    if "nc" not in _CACHE:
        _CACHE["nc"] = _build_nc()
    nc = _CACHE["nc"]
    in_maps = _host_inputs(
        np.asarray(query), np.asarray(key), np.asarray(value),
        np.asarray(Wq), np.asarray(Wk), np.asarray(Wv), np.asarray(Wo),
    )
    trace = os.environ.get("KERNEL_TRACE", "0") == "1"
    try:
        res = run_bass_kernel_spmd(
            nc, in_maps, core_ids=list(range(NCORES)), trace=trace
        )
    except ModuleNotFoundError:
        res = run_bass_kernel_spmd(
            nc, in_maps, core_ids=list(range(NCORES)), trace=False
        )
    LAST_RESULT = res
    out = np.concatenate([res.results[c]["out"] for c in range(NCORES)], axis=0)
    return out.reshape(1, S, D).astype(np.float32)


# revision 3
# speedup vs baseline: 1.0141x; 1.0141x over previous
"""LocalAttention (B=1, S=4096, D=1024, H=16, hd=64, window=128) on 8 trn2 cores.

Sequence-parallel: core c owns queries [512c, 512c+512), K/V halo of 768 rows.
All weights replicated bf16; fp32 PSUM accumulation.

v2 vs baseline:
  - attention processes 2 heads per "pack": one [128,768] fp32 PSUM score tile
    (2 banks; head A cols 0-383, head B 384-767), ONE exp per pack.
  - score r-chunks stored [r0|r2|r1] per head so the two masked triangle chunks
    are adjacent: ONE broadcast tensor_mul over [128,2,256] applies both heads'
    masks (middle all-ones chunk never multiplied).
  - softmax denominators via ones-column (VROW=65); per pack one DVE reciprocal
    [128,2] + one broadcast tensor_mul (PSUM->SBUF bf16) normalizes both heads.
  - software-pipelined flat pack loop: QK(pk) issues ahead, PV lags 2 packs so
    exp/mask latency never stalls the in-order PE queue; transpose + output
    projection of window t fill PE slack during packs 8t+10..8t+17.
  - 8 PE transposes of a q-block share one PSUM bank; ONE [128,8x128] copy
    evacuates them.
  - input DMAs ordered (qtin, per-eb wq slices first) so qproj starts ~3.5us.
"""

import os

import numpy as np
import ml_dtypes

import concourse.bass as bass
import concourse.bacc as bacc
import concourse.mybir as mybir
import concourse.tile as tile
from concourse.bass_utils import run_bass_kernel_spmd

BF16 = mybir.dt.bfloat16
FP32 = mybir.dt.float32

NCORES = 8
S = 4096
D = 1024
H = 16
HD = 64
E = H * HD  # 1024
WIN = 128
SL = S // NCORES       # 512 queries per core
SK = SL + 2 * WIN      # 768 keys/values incl. halo
NQB = SL // 128        # 4 query blocks
NKB = SK // 128        # 6 key blocks
NDB = D // 128         # 8 contraction blocks
NEB = E // 128         # 8 embed blocks
VROW = HD + 1          # 65: v columns per head incl. ones column
NPACK = NQB * NEB      # 32 flat (qb, head-pair) packs
PW = NPACK // NQB      # 8 packs per q-block window
ROFF = (0, 256, 128)   # col offset of r-chunk within a head's 384 block

_CACHE = {}
LAST_RESULT = None


def _build_nc():
    nc = bacc.Bacc("TRN2", target_bir_lowering=False, debug=False)

    qt_d = nc.dram_tensor("qt", [D, SL], BF16, kind="ExternalInput").ap()
    kt_d = nc.dram_tensor("kt", [D, SK], BF16, kind="ExternalInput").ap()
    vt_d = nc.dram_tensor("vt", [D, SK], BF16, kind="ExternalInput").ap()
    wq_d = nc.dram_tensor("wq", [D, E], BF16, kind="ExternalInput").ap()
    wk_d = nc.dram_tensor("wk", [D, E], BF16, kind="ExternalInput").ap()
    wv_d = nc.dram_tensor("wv", [D, E], BF16, kind="ExternalInput").ap()
    wo_d = nc.dram_tensor("wo", [E, D], BF16, kind="ExternalInput").ap()
    msk_d = nc.dram_tensor("msk", [NQB, 128, 256], BF16, kind="ExternalInput").ap()
    idn_d = nc.dram_tensor("idn", [128, 128], BF16, kind="ExternalInput").ap()
    out_d = nc.dram_tensor("out", [SL, D], FP32, kind="ExternalOutput").ap()

    with tile.TileContext(nc) as tc:
        pools = []

        def pool(name, bufs, **kw):
            p = tc.tile_pool(name=name, bufs=bufs, **kw)
            pools.append(p)
            return p.__enter__()

        const = pool("const", 1)
        ps_pool = pool("ps", 2, space="PSUM")        # 2 banks: proj/oproj/tr
        pscore = pool("pscore", 4, space="PSUM")     # 4x1 bank: score half-packs
        ppv_pool = pool("ppv", 2, space="PSUM")      # 2 banks: pv packs
        ep = pool("expp", 4)
        rp = pool("recip", 4)

        # ---- persistent SBUF tensors ----
        wq_sb = const.tile([128, NDB * E], BF16, tag="wq")
        wk_sb = const.tile([128, NDB * E], BF16, tag="wk")
        wv_sb = const.tile([128, NDB * E], BF16, tag="wv")
        wo_sb = const.tile([128, NEB * D], BF16, tag="wo")
        qtin_sb = const.tile([128, NDB * SL], BF16, tag="qtin")
        ktin_sb = const.tile([128, NDB * SK], BF16, tag="ktin")
        vtin_sb = const.tile([128, NDB * SK], BF16, tag="vtin")
        qt_sb = const.tile([128, NEB * SL], BF16, tag="qt")    # [e,s] per e-blk
        kt_sb = const.tile([128, NEB * SK], BF16, tag="kt")
        v_sb = const.tile([128, NKB * H * VROW], BF16, tag="v")  # [s, h*65] per k-blk
        msk_sb = const.tile([128, NQB * 256], BF16, tag="msk")
        idn_sb = const.tile([128, 128], BF16, tag="idn")
        ao_sb = const.tile([128, NQB * E], BF16, tag="ao")     # attn out [sq, e]
        aot_sb = const.tile([128, NEB * SL], BF16, tag="aot")  # transposed [e, sq]
        o_sb = const.tile([128, NQB * D], FP32, tag="o")

        sync = nc.sync

        # ---- input DMAs: whole tensors on the sync queue, dependency order.
        # (eb-sliced loads produce 256B descriptors, ~6x slower; and extra
        # dma_starts serialize on the limited DMA semaphore slots.)
        sync.dma_start(
            qtin_sb[:].rearrange("p (b e) -> p b e", e=SL),
            qt_d.rearrange("(b p) e -> p b e", p=128),
        )
        wq3 = wq_sb[:].rearrange("p (b e) -> p b e", e=E)
        wq3d = wq_d.rearrange("(b p) e -> p b e", p=128)
        sync.dma_start(wq3[:, :, 0:512], wq3d[:, :, 0:512])
        sync.dma_start(wq3[:, :, 512:1024], wq3d[:, :, 512:1024])
        sync.dma_start(
            vtin_sb[:].rearrange("p (b e) -> p b e", e=SK),
            vt_d.rearrange("(b p) e -> p b e", p=128),
        )
        sync.dma_start(
            wv_sb[:].rearrange("p (b e) -> p b e", e=E),
            wv_d.rearrange("(b p) e -> p b e", p=128),
        )
        sync.dma_start(
            ktin_sb[:].rearrange("p (b e) -> p b e", e=SK),
            kt_d.rearrange("(b p) e -> p b e", p=128),
        )
        sync.dma_start(
            wk_sb[:].rearrange("p (b e) -> p b e", e=E),
            wk_d.rearrange("(b p) e -> p b e", p=128),
        )
        sync.dma_start(
            wo_sb[:].rearrange("p (b e) -> p b e", e=D),
            wo_d.rearrange("(b p) e -> p b e", p=128),
        )
        # small late-needed tensors on the scalar queue
        nc.scalar.dma_start(
            msk_sb[:].rearrange("p (m c) -> p m c", c=256),
            msk_d.rearrange("m p c -> p m c"),
        )
        nc.scalar.dma_start(idn_sb[:], idn_d[:])

        # ones columns of v_sb (col hd=64 of each head group)
        v3 = v_sb[:].rearrange("p (k h c) -> p k h c", k=NKB, h=H)
        nc.gpsimd.memset(v3[:, :, :, HD:VROW], 1.0)

        # ---- q projection: qt[e,s] = Wq[d,e].T @ QT[d,s] ----
        for eb in range(NEB):
            ps = ps_pool.tile([128, 512], FP32, tag="ps", name=f"psq{eb}")
            for db in range(NDB):
                nc.tensor.matmul(
                    ps[:],
                    lhsT=wq_sb[:, db * E + eb * 128: db * E + (eb + 1) * 128],
                    rhs=qtin_sb[:, db * SL: db * SL + SL],
                    start=(db == 0),
                    stop=(db == NDB - 1),
                )
            nc.vector.tensor_copy(qt_sb[:, eb * SL:(eb + 1) * SL], ps[:])

        # ---- k projection for one e-block (two s-chunks) ----
        def emit_kproj_eb(eb):
            for s0, s1 in ((0, 512), (512, SK)):
                ps = ps_pool.tile([128, 512], FP32, tag="ps", name=f"psk{eb}_{s0}")
                for db in range(NDB):
                    nc.tensor.matmul(
                        ps[:, : s1 - s0],
                        lhsT=wk_sb[:, db * E + eb * 128: db * E + (eb + 1) * 128],
                        rhs=ktin_sb[:, db * SK + s0: db * SK + s1],
                        start=(db == 0),
                        stop=(db == NDB - 1),
                    )
                if s1 - s0 == 512:
                    nc.vector.tensor_copy(
                        kt_sb[:, eb * SK + s0: eb * SK + s1], ps[:, : s1 - s0]
                    )
                else:
                    nc.scalar.copy(
                        kt_sb[:, eb * SK + s0: eb * SK + s1], ps[:, : s1 - s0]
                    )

        # ---- v projection (natural) for one (kb, eh) group ----
        def emit_vproj_group(kb, eh):
            ps = ps_pool.tile([128, 512], FP32, tag="ps", name=f"psv{kb}_{eh}")
            for db in range(NDB):
                nc.tensor.matmul(
                    ps[:],
                    lhsT=vtin_sb[:, db * SK + kb * 128: db * SK + (kb + 1) * 128],
                    rhs=wv_sb[:, db * E + eh * 512: db * E + (eh + 1) * 512],
                    start=(db == 0),
                    stop=(db == NDB - 1),
                )
            dst = v3[:, kb, eh * 8:(eh + 1) * 8, 0:HD]
            src = ps[:].rearrange("p (h c) -> p h c", c=HD)
            nc.scalar.copy(dst, src)

        # v for the first attention window (kb 0..2) before the pack loop;
        # kb 3/4/5 are woven into windows 0/1/2 as PE filler.
        for kb in range(3):
            for eh in range(2):
                emit_vproj_group(kb, eh)

        # ---- attention: flat pack loop, PE software-pipelined ----
        scale = 1.0 / np.sqrt(HD)
        expps = {}
        ppvs = {}
        state = {}

        def emit_qk_exp(pk):
            qb, p = divmod(pk, PW)
            expp = ep.tile([128, 768], BF16, tag="expp", name=f"ex{pk}")
            for hcol, hp in ((0, 0), (1, 64)):
                base = hcol * 384
                pscr = pscore.tile(
                    [128, 384], FP32, tag="scr", name=f"scr{pk}_{hcol}"
                )
                kh = kt_sb[hp:hp + HD]
                qh = qt_sb[hp:hp + HD]
                for r in range(3):
                    kb = qb + r
                    nc.tensor.matmul(
                        pscr[:, ROFF[r]: ROFF[r] + 128],
                        lhsT=kh[:, p * SK + kb * 128: p * SK + (kb + 1) * 128],
                        rhs=qh[:, p * SL + qb * 128: p * SL + (qb + 1) * 128],
                        start=True,
                        stop=True,
                    )
                nc.scalar.activation(
                    expp[:, base: base + 384],
                    pscr[:],
                    mybir.ActivationFunctionType.Exp,
                    scale=scale,
                )
            expps[pk] = expp

        def emit_mask(pk):
            qb = pk // PW
            e3 = expps[pk][:].rearrange("p (h c) -> p h c", c=384)[:, :, 0:256]
            m = msk_sb[:, qb * 256:(qb + 1) * 256]
            mb = m.unsqueeze(1).broadcast_to((128, 2, 256))
            nc.vector.tensor_mul(e3, e3, mb)

        def emit_pv(pk):
            qb, p = divmod(pk, PW)
            expp = expps.pop(pk)
            ppv = ppv_pool.tile([128, 2 * VROW], FP32, tag="pv", name=f"pv{pk}")
            for hcol in range(2):
                h = 2 * p + hcol
                for r in range(3):
                    kb = qb + r
                    nc.tensor.matmul(
                        ppv[:, hcol * VROW:(hcol + 1) * VROW],
                        lhsT=expp[
                            :, hcol * 384 + ROFF[r]: hcol * 384 + ROFF[r] + 128
                        ],
                        rhs=v_sb[:, (kb * H + h) * VROW:(kb * H + h + 1) * VROW],
                        start=(r == 0),
                        stop=(r == 2),
                    )
            ppvs[pk] = ppv

        def emit_norm(pk):
            qb, p = divmod(pk, PW)
            ppv = ppvs.pop(pk)
            ppv3 = ppv[:].rearrange("p (h c) -> p h c", c=VROW)
            rcp = rp.tile([128, 2], FP32, tag="rcp", name=f"rcp{pk}")
            nc.vector.reciprocal(rcp[:], ppv3[:, :, HD])
            ao3 = ao_sb[
                :, qb * E + 2 * p * HD: qb * E + (2 * p + 2) * HD
            ].rearrange("p (h c) -> p h c", c=HD)
            rcp_b = rcp[:].unsqueeze(2).broadcast_to((128, 2, HD))
            nc.vector.tensor_mul(ao3, ppv3[:, :, 0:HD], rcp_b)

        def emit_oproj_mm(t, dh, ebs, slot_key):
            if (t, dh) not in state:
                state[(t, dh)] = ps_pool.tile(
                    [128, 512], FP32, tag="ps", name=f"po{t}_{dh}"
                )
            po = state[(t, dh)]
            for eb in ebs:
                nc.tensor.matmul(
                    po[:],
                    lhsT=aot_sb[:, eb * SL + t * 128: eb * SL + (t + 1) * 128],
                    rhs=wo_sb[:, eb * D + dh * 512: eb * D + (dh + 1) * 512],
                    start=(eb == 0),
                    stop=(eb == NEB - 1),
                )
            if ebs[-1] == NEB - 1:
                po = state.pop((t, dh))
                nc.vector.tensor_copy(
                    o_sb[:, t * D + dh * 512: t * D + (dh + 1) * 512], po[:]
                )

        def emit_filler(pk):
            """Fillers for target window t occupy packs 8t+10 .. 8t+17."""
            if pk < 10:
                return
            t, u = divmod(pk - 10, PW)
            if t >= NQB:
                return
            if u == 0 or u == 1:
                # 3 transposes each into the shared pt bank
                if u == 0:
                    state[("pt", t)] = ps_pool.tile(
                        [128, 1024], BF16, tag="ps", name=f"pt{t}"
                    )
                pt = state[("pt", t)]
                for eb in (range(0, 3) if u == 0 else range(3, 6)):
                    nc.tensor.transpose(
                        pt[:, eb * 128:(eb + 1) * 128],
                        ao_sb[:, t * E + eb * 128: t * E + (eb + 1) * 128],
                        idn_sb[:],
                    )
            elif u == 2:
                pt = state.pop(("pt", t))
                for eb in (6, 7):
                    nc.tensor.transpose(
                        pt[:, eb * 128:(eb + 1) * 128],
                        ao_sb[:, t * E + eb * 128: t * E + (eb + 1) * 128],
                        idn_sb[:],
                    )
                # one evacuation of all 8 transposed chunks
                nc.scalar.copy(
                    aot_sb[:]
                    .rearrange("p (b s) -> p b s", s=SL)[:, :, t * 128:(t + 1) * 128],
                    pt[:].rearrange("p (b s) -> p b s", s=128),
                )
            elif u == 3:
                emit_oproj_mm(t, 0, [0, 1, 2, 3], None)
            elif u == 4:
                emit_oproj_mm(t, 0, [4, 5, 6, 7], None)
            elif u == 5:
                emit_oproj_mm(t, 1, [0, 1, 2, 3], None)
            elif u == 6:
                emit_oproj_mm(t, 1, [4, 5, 6, 7], None)
            elif u == 7:
                sync.dma_start(
                    out_d[t * 128:(t + 1) * 128, :], o_sb[:, t * D:(t + 1) * D]
                )

        for pk in range(NPACK + 2):
            qb, p = divmod(pk, PW)
            if pk < NPACK:
                if qb == 0:
                    # kproj for this pack's head-pair, right before its QK
                    emit_kproj_eb(p)
                emit_qk_exp(pk)
            if pk >= 2 and pk - 2 < NPACK:
                emit_pv(pk - 2)
                emit_norm(pk - 2)
            if pk >= 1 and pk - 1 < NPACK:
                emit_mask(pk - 1)
            # weave remaining vproj groups: kb=qb+3, one group at p=1 and p=5
            if pk < NPACK and qb < 3 and p in (1, 5):
                emit_vproj_group(qb + 3, 0 if p == 1 else 1)
            emit_filler(pk)

        # tail: remaining fillers (windows whose slots extend past NPACK+1)
        for pk in range(NPACK + 2, NQB * PW + 10 + PW):
            emit_filler(pk)

        for p in reversed(pools):
            p.__exit__(None, None, None)

    nc.compile()
    return nc


def _host_inputs(query, key, value, Wq, Wk, Wv, Wo):
    bf = ml_dtypes.bfloat16
    q2 = np.ascontiguousarray(query.reshape(S, D))
    k2 = np.asarray(key).reshape(S, D)
    v2 = np.asarray(value).reshape(S, D)
    kpad = np.zeros((S + 2 * WIN, D), np.float32)
    kpad[WIN:WIN + S] = k2
    vpad = np.zeros((S + 2 * WIN, D), np.float32)
    vpad[WIN:WIN + S] = v2

    wq = np.ascontiguousarray(Wq.astype(bf))
    wk = np.ascontiguousarray(Wk.astype(bf))
    wv = np.ascontiguousarray(Wv.astype(bf))
    wo = np.ascontiguousarray(Wo.astype(bf))
    idn = np.eye(128, dtype=bf)

    kt = np.arange(128)[:, None]
    qi = np.arange(128)[None, :]
    tri0 = (qi <= kt).astype(bf)
    tri2 = (kt <= qi).astype(bf)
    zeros = np.zeros((128, 128), bf)

    in_maps = []
    for c in range(NCORES):
        s0 = c * SL
        qt = np.ascontiguousarray(q2[s0:s0 + SL].T.astype(bf))
        ktc = np.ascontiguousarray(kpad[s0:s0 + SK].T.astype(bf))
        vtc = np.ascontiguousarray(vpad[s0:s0 + SK].T.astype(bf))
        msk = np.empty((NQB, 128, 256), bf)
        for qb in range(NQB):
            m0 = zeros if (c == 0 and qb == 0) else tri0
            m2 = zeros if (c == NCORES - 1 and qb == NQB - 1) else tri2
            msk[qb] = np.hstack([m0, m2])
        in_maps.append({
            "qt": qt, "kt": ktc, "vt": vtc,
            "wq": wq, "wk": wk, "wv": wv, "wo": wo,
            "msk": msk, "idn": idn,
        })
    return in_maps


def kernel(query, key, value, Wq, Wk, Wv, Wo):
    global LAST_RESULT
    if "nc" not in _CACHE:
        _CACHE["nc"] = _build_nc()
    nc = _CACHE["nc"]
    in_maps = _host_inputs(
        np.asarray(query), np.asarray(key), np.asarray(value),
        np.asarray(Wq), np.asarray(Wk), np.asarray(Wv), np.asarray(Wo),
    )
    trace = os.environ.get("KERNEL_TRACE", "0") == "1"
    try:
        res = run_bass_kernel_spmd(
            nc, in_maps, core_ids=list(range(NCORES)), trace=trace
        )
    except ModuleNotFoundError:
        res = run_bass_kernel_spmd(
            nc, in_maps, core_ids=list(range(NCORES)), trace=False
        )
    LAST_RESULT = res
    out = np.concatenate([res.results[c]["out"] for c in range(NCORES)], axis=0)
    return out.reshape(1, S, D).astype(np.float32)
